# revision 6
# baseline (speedup 1.0000x reference)
"""MeanAggregator (GNN mean message passing) on 8 Trainium2 NeuronCores.

reference:
    neigh_feats = features[neigh_idx]          # [batch, num_sample, d_feat]
    out = mean(neigh_feats, axis=1)            # [batch, d_feat]

Shapes (hardcoded): features [1_000_000, 128] f32, neigh_idx [100_000, 16] i64.

Data-parallel over the batch across 8 cores (12_500 rows each), features
replicated (host-cast to bf16: rel tolerance 2e-2 >> bf16 error).

Default path = radix-staged bulk gather ("x4"): the only bulk-descriptor
gather primitive (InstDMAGatherAnt / gpsimd.dma_gather) takes int16 indices
(reach 32767 rows x 256 B), so the 256 MB bf16 table is covered by 31
"plane" calls -- which scrambles output order (each call writes consecutive
SBUF slots). A 2-pass radix fixes the order with all-static capacities:

  h1 (per plane): dma_gather the plane's sampled rows in REGION-sorted order
      (region = fixed run of batch tiles; fixed-capacity per-(plane, region)
      sections, dummy-padded), then one static HWDGE DMA into a DRAM staging
      area laid out region-major.
  h2 (per 2-tile chunk): dma_gather from that region's staging block
      (<= 32767 rows, int16-addressable) in exact batch-interleaved order:
      token q' = (tile_local*16 + s)*128 + p lands at partition p, col
      tile_local*16 + s, so one batch row's 16 samples are 16 consecutive
      256 B slots on its own partition. DVE tree-reduce, scale 1/16 with f32
      output, DMA out.

~1 us SWDGE fixed cost is paid ~80x per rep instead of 1568x (the per-sample
indirect-DMA fallback below, used only on section-capacity overflow --
P < 1e-6 for uniform indices -- or if the x4 build fails).
"""

import numpy as np
import ml_dtypes

import concourse.bacc as bacc
import concourse.bass as bass
import concourse.mybir as mybir
import concourse.tile as tile
from concourse.library_config import mlp
from concourse.bass_utils import run_bass_kernel_spmd

N_CORES = 8
P = 128
D = 128
S = 16
NUM_NODES = 1_000_000
BATCH = 100_000
B_CORE = BATCH // N_CORES            # 12500
NTILES = (B_CORE + P - 1) // P       # 98

PLANE_ROWS = 32768                   # int16 reach per dma_gather call
PLANES = (NUM_NODES + PLANE_ROWS - 1) // PLANE_ROWS   # 31
REGION_TILES = [12] * 8 + [2]        # sum = 98, all even (CH divides each)
NREG = len(REGION_TILES)
SEC_CAP = 1024                       # tokens per (plane, region) section
SEC_COLS = SEC_CAP // P              # 8
H1CAP = NREG * SEC_CAP               # 9216 tokens per plane call
REG_ROWS = PLANES * SEC_CAP          # 31744 staging rows/region (<= 32767)
CH = 2                               # tiles per h2 call
CHTOK = CH * P * S                   # 4096


class CapacityError(Exception):
    pass


# ---------------------------------------------------------------- host prep

def _wrap16(lists):
    """[ncalls, n] int16 logical lists -> [128, ncalls*(n//16)]: entry j of
    call k at (partition j%16, col k*(n//16)+j//16), replicated across the 8
    groups of 16 partitions."""
    ncalls, n = lists.shape
    w = lists.reshape(ncalls, n // 16, 16).transpose(0, 2, 1)
    out = np.concatenate([w[k] for k in range(ncalls)], axis=1)
    return np.ascontiguousarray(np.tile(out, (8, 1)))


def _prep_x4(idx32):
    """idx32 [NTILES*P, S] int32 -> (h1_idx, h2_idx) wrapped int16 tensors."""
    assert idx32.shape == (NTILES * P, S)
    n = idx32.reshape(-1)
    j = np.repeat(np.arange(NTILES * P) // P, S)
    p = np.repeat(np.arange(NTILES * P) % P, S)
    s = np.tile(np.arange(S), NTILES * P)

    reg_of_tile = np.repeat(np.arange(NREG), REGION_TILES)
    tile0_of_reg = np.concatenate(([0], np.cumsum(REGION_TILES)[:-1]))
    r = reg_of_tile[j]
    plane = n >> 15
    local = (n & 32767).astype(np.int16)

    sec = plane * NREG + r
    order = np.argsort(sec, kind="stable")
    counts = np.bincount(sec, minlength=PLANES * NREG)
    if counts.max() > SEC_CAP:
        raise CapacityError(f"section overflow: {counts.max()} > {SEC_CAP}")
    starts = np.concatenate(([0], np.cumsum(counts)[:-1]))
    rank = np.empty_like(order)
    rank[order] = np.arange(order.size) - starts[sec[order]]

    h1 = np.zeros((PLANES, H1CAP), dtype=np.int16)
    h1[plane, r * SEC_CAP + rank] = local

    srow = (plane * SEC_CAP + (rank % P) * SEC_COLS + rank // P).astype(
        np.int16)
    qpos = ((j - tile0_of_reg[r]) * S + s) * P + p
    h2_list = np.zeros(NTILES * P * S, dtype=np.int16)
    reg_tok0 = np.concatenate(
        ([0], np.cumsum(np.array(REGION_TILES) * P * S)[:-1]))
    h2_list[reg_tok0[r] + qpos] = srow
    return _wrap16(h1), _wrap16(h2_list.reshape(1, -1))


def _pad_core(idx32_core):
    pad = NTILES * P - idx32_core.shape[0]
    if pad:
        idx32_core = np.concatenate([idx32_core, idx32_core[:pad]], axis=0)
    return idx32_core


def _host_prep(neigh_idx):
    idx32 = np.asarray(neigh_idx).astype(np.int32)
    return [
        _prep_x4(_pad_core(idx32[c * B_CORE : (c + 1) * B_CORE]))
        for c in range(N_CORES)
    ]


def _prep_idx(idx32):
    """per-sample fallback layout: [b_core, S] -> [P, ntiles*S] int32."""
    b_core = idx32.shape[0]
    ntiles = (b_core + P - 1) // P
    pad = ntiles * P - b_core
    if pad:
        idx32 = np.concatenate([idx32, idx32[:pad]], axis=0)
    return np.ascontiguousarray(
        idx32.reshape(ntiles, P, S).transpose(1, 0, 2).reshape(P, ntiles * S)
    )


# ------------------------------------------------------------ x4 bass kernel

def build_nc_x4(reps=1, g1bufs=2, g2bufs=3):
    nc = bacc.Bacc("TRN2", target_bir_lowering=False, num_swdge_queues=1)
    feats = nc.dram_tensor("features", [NUM_NODES, D], mybir.dt.bfloat16,
                           kind="ExternalInput")
    h1i = nc.dram_tensor("h1_idx", [P, PLANES * (H1CAP // 16)],
                         mybir.dt.int16, kind="ExternalInput")
    h2i = nc.dram_tensor("h2_idx", [P, NTILES * P * S // 16],
                         mybir.dt.int16, kind="ExternalInput")
    out = nc.dram_tensor("out", [NTILES * P, D], mybir.dt.float32,
                         kind="ExternalOutput")
    staging = nc.dram_tensor("staging", [NREG * REG_ROWS, D],
                             mybir.dt.bfloat16, kind="Internal")
    out_re = out[:].rearrange("(j p) d -> p j d", p=P)
    st_w = staging[:].rearrange(
        "(r pl p c) d -> pl p r (c d)", r=NREG, pl=PLANES, p=P, c=SEC_COLS)

    reg_tile0 = np.concatenate(([0], np.cumsum(REGION_TILES)[:-1]))

    with tile.TileContext(nc) as tc:
        with (
            tc.tile_pool(name="idxp", bufs=1) as idxp,
            tc.tile_pool(name="g1p", bufs=g1bufs) as g1p,
            tc.tile_pool(name="g2p", bufs=g2bufs) as g2p,
            tc.tile_pool(name="resp", bufs=g2bufs) as resp,
        ):
            h1_sb = idxp.tile([P, PLANES * (H1CAP // 16)], mybir.dt.int16)
            h2_sb = idxp.tile([P, NTILES * P * S // 16], mybir.dt.int16)
            nc.sync.dma_start(out=h1_sb[:], in_=h1i[:])
            nc.sync.dma_start(out=h2_sb[:], in_=h2i[:])
            probe = idxp.tile([1, 1], mybir.dt.int16)
            nc.gpsimd.load_library(mlp)
            nc.gpsimd.tensor_copy(probe[:], h1_sb[:1, :1])
            nc.gpsimd.tensor_copy(probe[:], h2_sb[:1, :1])

            def body():
                # phase 1: plane gathers -> region-major staging
                for pl in range(PLANES):
                    g1 = g1p.tile([P, (H1CAP // P) * D], mybir.dt.bfloat16)
                    nc.gpsimd.memset(g1[:1, :1], 0)
                    g1_3 = g1[:].rearrange("p (i d) -> p i d",
                                           i=H1CAP // P, d=D)
                    lo = pl * PLANE_ROWS
                    hi = min(NUM_NODES, lo + PLANE_ROWS)
                    nc.gpsimd.dma_gather(
                        g1_3, feats[lo:hi, :],
                        h1_sb[:, pl * (H1CAP // 16) : (pl + 1) * (H1CAP // 16)],
                        H1CAP, H1CAP, D, queue_num=0,
                        single_packet=False,
                    )
                    g1_w = g1[:].rearrange("p (r c d) -> p r (c d)",
                                           r=NREG, c=SEC_COLS, d=D)
                    nc.sync.dma_start(out=st_w[pl], in_=g1_w)

                # phase 2: per-chunk batch-ordered gathers + reduce
                for ci in range(NTILES // CH):
                    j0 = ci * CH
                    reg = int(np.searchsorted(reg_tile0, j0, side="right")) - 1
                    g2 = g2p.tile([P, CH * S * D], mybir.dt.bfloat16)
                    nc.gpsimd.memset(g2[:1, :1], 0)
                    g2_3 = g2[:].rearrange("p (i d) -> p i d", i=CH * S, d=D)
                    nc.gpsimd.dma_gather(
                        g2_3,
                        staging[reg * REG_ROWS : (reg + 1) * REG_ROWS, :],
                        h2_sb[:, ci * (CHTOK // 16) : (ci + 1) * (CHTOK // 16)],
                        CHTOK, CHTOK, D, queue_num=0,
                        single_packet=False,
                    )
                    g4 = g2[:].rearrange("p (c s d) -> p c s d",
                                         c=CH, s=S, d=D)
                    width = S
                    while width > 1:
                        half = width // 2
                        nc.vector.tensor_add(
                            g4[:, :, 0:half, :],
                            g4[:, :, 0:half, :],
                            g4[:, :, half : 2 * half, :],
                        )
                        width = half
                    r = resp.tile([P, CH * D], mybir.dt.float32)
                    r4 = r[:].rearrange("p (c o d) -> p c o d",
                                        c=CH, o=1, d=D)
                    nc.vector.tensor_scalar_mul(
                        r4[:, :, :, :], g4[:, :, 0:1, :], 1.0 / S)
                    nc.sync.dma_start(out=out_re[:, j0 : j0 + CH, :],
                                      in_=r[:])

            if reps == 1:
                body()
            else:
                with tc.For_i(0, reps, 1):
                    body()
    nc.compile()
    return nc


# --------------------------------------------- per-sample fallback kernel

def build_nc_persample(reps=1, n_queues=4):
    ntiles = NTILES
    nc = bacc.Bacc("TRN2", target_bir_lowering=False,
                   num_swdge_queues=n_queues)
    feats = nc.dram_tensor("features", [NUM_NODES, D], mybir.dt.float32,
                           kind="ExternalInput")
    idx = nc.dram_tensor("idx_t", [P, ntiles * S], mybir.dt.int32,
                         kind="ExternalInput")
    out = nc.dram_tensor("out", [ntiles * P, D], mybir.dt.float32,
                         kind="ExternalOutput")
    out_re = out[:].rearrange("(j p) d -> p j d", p=P)

    with tile.TileContext(nc) as tc:
        with (
            tc.tile_pool(name="idxp", bufs=1) as idxp,
            tc.tile_pool(name="gatp", bufs=3) as gatp,
            tc.tile_pool(name="resp", bufs=3) as resp,
        ):
            idx_sb = idxp.tile([P, ntiles * S], mybir.dt.int32)
            nc.sync.dma_start(out=idx_sb[:], in_=idx[:])
            probe = idxp.tile([1, 1], mybir.dt.int32)
            nc.gpsimd.tensor_copy(probe[:], idx_sb[:1, :1])

            def tile_body(j):
                g = gatp.tile([P, S * D], mybir.dt.float32)
                nc.gpsimd.memset(g[:1, :1], 0)
                for s in range(S):
                    inst = nc.gpsimd.indirect_dma_start(
                        out=g[:, s * D : (s + 1) * D],
                        out_offset=None,
                        in_=feats[:],
                        in_offset=bass.IndirectOffsetOnAxis(
                            ap=idx_sb[:, j * S + s : j * S + s + 1], axis=0
                        ),
                    )
                    if n_queues > 1:
                        inst.ins.queue = f"qPoolDynamic{s % n_queues or ''}"
                width = S
                while width > 1:
                    half = width // 2
                    nc.vector.tensor_add(
                        g[:, 0 : half * D],
                        g[:, 0 : half * D],
                        g[:, half * D : 2 * half * D],
                    )
                    width = half
                r = resp.tile([P, D], mybir.dt.float32)
                nc.vector.tensor_scalar_mul(r[:], g[:, 0:D], 1.0 / S)
                nc.sync.dma_start(out=out_re[:, j, :], in_=r[:])

            if reps == 1:
                for j in range(ntiles):
                    tile_body(j)
            else:
                with tc.For_i(0, reps, 1):
                    for j in range(ntiles):
                        tile_body(j)
    nc.compile()
    return nc


_nc_cache = {}


def _get(builder, **kw):
    key = (builder.__name__, tuple(sorted(kw.items())))
    if key not in _nc_cache:
        _nc_cache[key] = builder(**kw)
    return _nc_cache[key]


def _run_x4(features, neigh_idx):
    prep = _host_prep(neigh_idx)          # raises CapacityError on overflow
    feats_bf = np.ascontiguousarray(
        np.asarray(features, dtype=np.float32).astype(ml_dtypes.bfloat16))
    nc = _get(build_nc_x4)
    in_maps = [
        {"features": feats_bf, "h1_idx": prep[c][0], "h2_idx": prep[c][1]}
        for c in range(N_CORES)
    ]
    res = run_bass_kernel_spmd(nc, in_maps, core_ids=list(range(N_CORES)))
    return np.concatenate([r["out"][:B_CORE] for r in res.results], axis=0)


def _run_persample(features, neigh_idx):
    features = np.ascontiguousarray(features, dtype=np.float32)
    idx32 = np.asarray(neigh_idx).astype(np.int32)
    nc = _get(build_nc_persample)
    in_maps = [
        {
            "features": features,
            "idx_t": _prep_idx(idx32[c * B_CORE : (c + 1) * B_CORE]),
        }
        for c in range(N_CORES)
    ]
    res = run_bass_kernel_spmd(nc, in_maps, core_ids=list(range(N_CORES)))
    return np.concatenate([r["out"][:B_CORE] for r in res.results], axis=0)


def kernel(features, neigh_idx, num_sample):
    assert np.asarray(features).shape == (NUM_NODES, D)
    assert np.asarray(neigh_idx).shape == (BATCH, S)
    try:
        return _run_x4(features, neigh_idx)
    except Exception as e:  # CapacityError or any x4 build/run failure
        import sys
        print(f"x4 path failed ({type(e).__name__}: {e}); "
              "falling back to per-sample kernel", file=sys.stderr)
        return _run_persample(features, neigh_idx)


# revision 7
# speedup vs baseline: 1.2380x; 1.2380x over previous
"""MeanAggregator (GNN mean message passing) on 8 Trainium2 NeuronCores.

reference:
    neigh_feats = features[neigh_idx]          # [batch, num_sample, d_feat]
    out = mean(neigh_feats, axis=1)            # [batch, d_feat]

Shapes (hardcoded): features [1_000_000, 128] f32, neigh_idx [100_000, 16] i64.

Data-parallel over the batch across 8 cores (12_500 rows each), features
replicated (host-cast to bf16: rel tolerance 2e-2 >> bf16 error).

Default path = radix-staged bulk gather ("x4"): the only bulk-descriptor
gather primitive (InstDMAGatherAnt / gpsimd.dma_gather) takes int16 indices
(reach 32767 rows x 256 B), so the 256 MB bf16 table is covered by 31
"plane" calls -- which scrambles output order (each call writes consecutive
SBUF slots). A 2-pass radix fixes the order with all-static capacities:

  h1 (per plane): dma_gather the plane's sampled rows in REGION-sorted order
      (region = fixed run of batch tiles; fixed-capacity per-(plane, region)
      sections, dummy-padded), then one static HWDGE DMA into a DRAM staging
      area laid out region-major.
  h2 (per 2-tile chunk): dma_gather from that region's staging block
      (<= 32767 rows, int16-addressable) in exact batch-interleaved order:
      token q' = (tile_local*16 + s)*128 + p lands at partition p, col
      tile_local*16 + s, so one batch row's 16 samples are 16 consecutive
      256 B slots on its own partition. DVE tree-reduce, scale 1/16 with f32
      output, DMA out.

~1 us SWDGE fixed cost is paid ~80x per rep instead of 1568x (the per-sample
indirect-DMA fallback below, used only on section-capacity overflow --
P < 1e-6 for uniform indices -- or if the x4 build fails).
"""

import numpy as np
import ml_dtypes

import concourse.bacc as bacc
import concourse.bass as bass
import concourse.mybir as mybir
import concourse.tile as tile
from concourse.library_config import mlp
from concourse.bass_utils import run_bass_kernel_spmd

N_CORES = 8
P = 128
D = 128
S = 16
NUM_NODES = 1_000_000
BATCH = 100_000
B_CORE = BATCH // N_CORES            # 12500
NTILES = (B_CORE + P - 1) // P       # 98

PLANE_ROWS = 32768                   # int16 reach per dma_gather call
PLANES = (NUM_NODES + PLANE_ROWS - 1) // PLANE_ROWS   # 31
REGION_TILES = [12] * 8 + [2]        # sum = 98, all even (CH divides each)
NREG = len(REGION_TILES)
SEC_CAP = 1024                       # tokens per (plane, region) section
SEC_COLS = SEC_CAP // P              # 8
H1CAP = NREG * SEC_CAP               # 9216 tokens per plane call
REG_ROWS = PLANES * SEC_CAP          # 31744 staging rows/region (<= 32767)
CH = 2                               # tiles per h2 call
CHTOK = CH * P * S                   # 4096


class CapacityError(Exception):
    pass


# ---------------------------------------------------------------- host prep

def _wrap16(lists):
    """[ncalls, n] int16 logical lists -> [128, ncalls*(n//16)]: entry j of
    call k at (partition j%16, col k*(n//16)+j//16), replicated across the 8
    groups of 16 partitions."""
    ncalls, n = lists.shape
    w = lists.reshape(ncalls, n // 16, 16).transpose(0, 2, 1)
    out = np.concatenate([w[k] for k in range(ncalls)], axis=1)
    return np.ascontiguousarray(np.tile(out, (8, 1)))


def _prep_x4(idx32):
    """idx32 [NTILES*P, S] int32 -> (h1_idx, h2_idx) wrapped int16 tensors."""
    assert idx32.shape == (NTILES * P, S)
    n = idx32.reshape(-1)
    j = np.repeat(np.arange(NTILES * P) // P, S)
    p = np.repeat(np.arange(NTILES * P) % P, S)
    s = np.tile(np.arange(S), NTILES * P)

    reg_of_tile = np.repeat(np.arange(NREG), REGION_TILES)
    tile0_of_reg = np.concatenate(([0], np.cumsum(REGION_TILES)[:-1]))
    r = reg_of_tile[j]
    plane = n >> 15
    local = (n & 32767).astype(np.int16)

    sec = plane * NREG + r
    order = np.argsort(sec, kind="stable")
    counts = np.bincount(sec, minlength=PLANES * NREG)
    if counts.max() > SEC_CAP:
        raise CapacityError(f"section overflow: {counts.max()} > {SEC_CAP}")
    starts = np.concatenate(([0], np.cumsum(counts)[:-1]))
    rank = np.empty_like(order)
    rank[order] = np.arange(order.size) - starts[sec[order]]

    h1 = np.zeros((PLANES, H1CAP), dtype=np.int16)
    h1[plane, r * SEC_CAP + rank] = local

    srow = (plane * SEC_CAP + (rank % P) * SEC_COLS + rank // P).astype(
        np.int16)
    qpos = ((j - tile0_of_reg[r]) * S + s) * P + p
    h2_list = np.zeros(NTILES * P * S, dtype=np.int16)
    reg_tok0 = np.concatenate(
        ([0], np.cumsum(np.array(REGION_TILES) * P * S)[:-1]))
    h2_list[reg_tok0[r] + qpos] = srow
    return _wrap16(h1), _wrap16(h2_list.reshape(1, -1))


def _pad_core(idx32_core):
    pad = NTILES * P - idx32_core.shape[0]
    if pad:
        idx32_core = np.concatenate([idx32_core, idx32_core[:pad]], axis=0)
    return idx32_core


def _host_prep(neigh_idx):
    idx32 = np.asarray(neigh_idx).astype(np.int32)
    return [
        _prep_x4(_pad_core(idx32[c * B_CORE : (c + 1) * B_CORE]))
        for c in range(N_CORES)
    ]


def _prep_idx(idx32):
    """per-sample fallback layout: [b_core, S] -> [P, ntiles*S] int32."""
    b_core = idx32.shape[0]
    ntiles = (b_core + P - 1) // P
    pad = ntiles * P - b_core
    if pad:
        idx32 = np.concatenate([idx32, idx32[:pad]], axis=0)
    return np.ascontiguousarray(
        idx32.reshape(ntiles, P, S).transpose(1, 0, 2).reshape(P, ntiles * S)
    )


# ------------------------------------------------------------ x4 bass kernel

def build_nc_x4(reps=1, g1bufs=2, g2bufs=3):
    nc = bacc.Bacc("TRN2", target_bir_lowering=False, num_swdge_queues=1)
    feats = nc.dram_tensor("features", [NUM_NODES, D], mybir.dt.bfloat16,
                           kind="ExternalInput")
    h1i = nc.dram_tensor("h1_idx", [P, PLANES * (H1CAP // 16)],
                         mybir.dt.int16, kind="ExternalInput")
    h2i = nc.dram_tensor("h2_idx", [P, NTILES * P * S // 16],
                         mybir.dt.int16, kind="ExternalInput")
    out = nc.dram_tensor("out", [NTILES * P, D], mybir.dt.float32,
                         kind="ExternalOutput")
    staging = nc.dram_tensor("staging", [NREG * REG_ROWS, D],
                             mybir.dt.bfloat16, kind="Internal")
    out_re = out[:].rearrange("(j p) d -> p j d", p=P)
    st_w = staging[:].rearrange(
        "(r pl p c) d -> pl p r (c d)", r=NREG, pl=PLANES, p=P, c=SEC_COLS)

    reg_tile0 = np.concatenate(([0], np.cumsum(REGION_TILES)[:-1]))

    with tile.TileContext(nc) as tc:
        with (
            tc.tile_pool(name="idxp", bufs=1) as idxp,
            tc.tile_pool(name="g1p", bufs=g1bufs) as g1p,
            tc.tile_pool(name="g2p", bufs=g2bufs) as g2p,
            tc.tile_pool(name="resp", bufs=g2bufs) as resp,
        ):
            h1_sb = idxp.tile([P, PLANES * (H1CAP // 16)], mybir.dt.int16)
            h2_sb = idxp.tile([P, NTILES * P * S // 16], mybir.dt.int16)
            nc.sync.dma_start(out=h1_sb[:], in_=h1i[:])
            nc.sync.dma_start(out=h2_sb[:], in_=h2i[:])
            probe = idxp.tile([1, 1], mybir.dt.int16)
            nc.gpsimd.load_library(mlp)
            nc.gpsimd.tensor_copy(probe[:], h1_sb[:1, :1])
            nc.gpsimd.tensor_copy(probe[:], h2_sb[:1, :1])

            def body():
                # phase 1: plane gathers -> region-major staging
                for pl in range(PLANES):
                    g1 = g1p.tile([P, (H1CAP // P) * D], mybir.dt.bfloat16)
                    nc.gpsimd.memset(g1[:1, :1], 0)
                    g1_3 = g1[:].rearrange("p (i d) -> p i d",
                                           i=H1CAP // P, d=D)
                    lo = pl * PLANE_ROWS
                    hi = min(NUM_NODES, lo + PLANE_ROWS)
                    nc.gpsimd.dma_gather(
                        g1_3, feats[lo:hi, :],
                        h1_sb[:, pl * (H1CAP // 16) : (pl + 1) * (H1CAP // 16)],
                        H1CAP, H1CAP, D, queue_num=0,
                        single_packet=False,
                    )
                    g1_w = g1[:].rearrange("p (r c d) -> p r (c d)",
                                           r=NREG, c=SEC_COLS, d=D)
                    nc.sync.dma_start(out=st_w[pl], in_=g1_w)

                # phase 2: per-chunk batch-ordered gathers + reduce
                for ci in range(NTILES // CH):
                    j0 = ci * CH
                    reg = int(np.searchsorted(reg_tile0, j0, side="right")) - 1
                    g2 = g2p.tile([P, CH * S * D], mybir.dt.bfloat16)
                    nc.gpsimd.memset(g2[:1, :1], 0)
                    g2_3 = g2[:].rearrange("p (i d) -> p i d", i=CH * S, d=D)
                    nc.gpsimd.dma_gather(
                        g2_3,
                        staging[reg * REG_ROWS : (reg + 1) * REG_ROWS, :],
                        h2_sb[:, ci * (CHTOK // 16) : (ci + 1) * (CHTOK // 16)],
                        CHTOK, CHTOK, D, queue_num=0,
                        single_packet=False,
                    )
                    g4 = g2[:].rearrange("p (c s d) -> p c s d",
                                         c=CH, s=S, d=D)
                    width = S
                    while width > 1:
                        half = width // 2
                        nc.vector.tensor_add(
                            g4[:, :, 0:half, :],
                            g4[:, :, 0:half, :],
                            g4[:, :, half : 2 * half, :],
                        )
                        width = half
                    r = resp.tile([P, CH * D], mybir.dt.float32)
                    r4 = r[:].rearrange("p (c o d) -> p c o d",
                                        c=CH, o=1, d=D)
                    nc.vector.tensor_scalar_mul(
                        r4[:, :, :, :], g4[:, :, 0:1, :], 1.0 / S)
                    nc.sync.dma_start(out=out_re[:, j0 : j0 + CH, :],
                                      in_=r[:])

            if reps == 1:
                body()
            else:
                with tc.For_i(0, reps, 1):
                    body()
    nc.compile()
    return nc


# --------------------------------------------- per-sample fallback kernel

def build_nc_persample(reps=1, n_queues=4):
    ntiles = NTILES
    nc = bacc.Bacc("TRN2", target_bir_lowering=False,
                   num_swdge_queues=n_queues)
    feats = nc.dram_tensor("features", [NUM_NODES, D], mybir.dt.float32,
                           kind="ExternalInput")
    idx = nc.dram_tensor("idx_t", [P, ntiles * S], mybir.dt.int32,
                         kind="ExternalInput")
    out = nc.dram_tensor("out", [ntiles * P, D], mybir.dt.float32,
                         kind="ExternalOutput")
    out_re = out[:].rearrange("(j p) d -> p j d", p=P)

    with tile.TileContext(nc) as tc:
        with (
            tc.tile_pool(name="idxp", bufs=1) as idxp,
            tc.tile_pool(name="gatp", bufs=3) as gatp,
            tc.tile_pool(name="resp", bufs=3) as resp,
        ):
            idx_sb = idxp.tile([P, ntiles * S], mybir.dt.int32)
            nc.sync.dma_start(out=idx_sb[:], in_=idx[:])
            probe = idxp.tile([1, 1], mybir.dt.int32)
            nc.gpsimd.tensor_copy(probe[:], idx_sb[:1, :1])

            def tile_body(j):
                g = gatp.tile([P, S * D], mybir.dt.float32)
                nc.gpsimd.memset(g[:1, :1], 0)
                for s in range(S):
                    inst = nc.gpsimd.indirect_dma_start(
                        out=g[:, s * D : (s + 1) * D],
                        out_offset=None,
                        in_=feats[:],
                        in_offset=bass.IndirectOffsetOnAxis(
                            ap=idx_sb[:, j * S + s : j * S + s + 1], axis=0
                        ),
                    )
                    if n_queues > 1:
                        inst.ins.queue = f"qPoolDynamic{s % n_queues or ''}"
                width = S
                while width > 1:
                    half = width // 2
                    nc.vector.tensor_add(
                        g[:, 0 : half * D],
                        g[:, 0 : half * D],
                        g[:, half * D : 2 * half * D],
                    )
                    width = half
                r = resp.tile([P, D], mybir.dt.float32)
                nc.vector.tensor_scalar_mul(r[:], g[:, 0:D], 1.0 / S)
                nc.sync.dma_start(out=out_re[:, j, :], in_=r[:])

            if reps == 1:
                for j in range(ntiles):
                    tile_body(j)
            else:
                with tc.For_i(0, reps, 1):
                    for j in range(ntiles):
                        tile_body(j)
    nc.compile()
    return nc


_nc_cache = {}


def _get(builder, **kw):
    key = (builder.__name__, tuple(sorted(kw.items())))
    if key not in _nc_cache:
        _nc_cache[key] = builder(**kw)
    return _nc_cache[key]


def _run_x4(features, neigh_idx):
    prep = _host_prep(neigh_idx)          # raises CapacityError on overflow
    feats_bf = np.ascontiguousarray(
        np.asarray(features, dtype=np.float32).astype(ml_dtypes.bfloat16))
    nc = _get(build_nc_x4)
    in_maps = [
        {"features": feats_bf, "h1_idx": prep[c][0], "h2_idx": prep[c][1]}
        for c in range(N_CORES)
    ]
    res = run_bass_kernel_spmd(nc, in_maps, core_ids=list(range(N_CORES)))
    return np.concatenate([r["out"][:B_CORE] for r in res.results], axis=0)


def _run_persample(features, neigh_idx):
    features = np.ascontiguousarray(features, dtype=np.float32)
    idx32 = np.asarray(neigh_idx).astype(np.int32)
    nc = _get(build_nc_persample)
    in_maps = [
        {
            "features": features,
            "idx_t": _prep_idx(idx32[c * B_CORE : (c + 1) * B_CORE]),
        }
        for c in range(N_CORES)
    ]
    res = run_bass_kernel_spmd(nc, in_maps, core_ids=list(range(N_CORES)))
    return np.concatenate([r["out"][:B_CORE] for r in res.results], axis=0)


def kernel(features, neigh_idx, num_sample):
    assert np.asarray(features).shape == (NUM_NODES, D)
    assert np.asarray(neigh_idx).shape == (BATCH, S)
    return _run_persample(features, neigh_idx)


# revision 8
# speedup vs baseline: 2.1841x; 1.7642x over previous
"""MeanAggregator (GNN mean message passing) on 8 Trainium2 NeuronCores.

reference:
    neigh_feats = features[neigh_idx]          # [batch, num_sample, d_feat]
    out = mean(neigh_feats, axis=1)            # [batch, d_feat]

Shapes (hardcoded): features [1_000_000, 128] f32, neigh_idx [100_000, 16] i64.

Data-parallel over the batch across 8 cores (12_500 rows each), features
replicated (host-cast to bf16: rel tolerance 2e-2 >> bf16 error).

Default path = radix-staged bulk gather ("x4"): the only bulk-descriptor
gather primitive (InstDMAGatherAnt / gpsimd.dma_gather) takes int16 indices
(reach 32767 rows x 256 B), so the 256 MB bf16 table is covered by 31
"plane" calls -- which scrambles output order (each call writes consecutive
SBUF slots). A 2-pass radix fixes the order with all-static capacities:

  h1 (per plane): dma_gather the plane's sampled rows in REGION-sorted order
      (region = fixed run of batch tiles; fixed-capacity per-(plane, region)
      sections, dummy-padded), then one static HWDGE DMA into a DRAM staging
      area laid out region-major.
  h2 (per 2-tile chunk): dma_gather from that region's staging block
      (<= 32767 rows, int16-addressable) in exact batch-interleaved order:
      token q' = (tile_local*16 + s)*128 + p lands at partition p, col
      tile_local*16 + s, so one batch row's 16 samples are 16 consecutive
      256 B slots on its own partition. DVE tree-reduce, scale 1/16 with f32
      output, DMA out.

~1 us SWDGE fixed cost is paid ~80x per rep instead of 1568x (the per-sample
indirect-DMA fallback below, used only on section-capacity overflow --
P < 1e-6 for uniform indices -- or if the x4 build fails).
"""

import numpy as np
import ml_dtypes

import concourse.bacc as bacc
import concourse.bass as bass
import concourse.mybir as mybir
import concourse.tile as tile
from concourse.library_config import mlp
from concourse.bass_utils import run_bass_kernel_spmd

N_CORES = 8
P = 128
D = 128
S = 16
NUM_NODES = 1_000_000
BATCH = 100_000
B_CORE = BATCH // N_CORES            # 12500
NTILES = (B_CORE + P - 1) // P       # 98

PLANE_ROWS = 32768                   # int16 reach per dma_gather call
PLANES = (NUM_NODES + PLANE_ROWS - 1) // PLANE_ROWS   # 31
REGION_TILES = [12] * 8 + [2]        # sum = 98, all even (CH divides each)
NREG = len(REGION_TILES)
SEC_CAP = 1024                       # tokens per (plane, region) section
SEC_COLS = SEC_CAP // P              # 8
H1CAP = NREG * SEC_CAP               # 9216 tokens per plane call
REG_ROWS = PLANES * SEC_CAP          # 31744 staging rows/region (<= 32767)
CH = 2                               # tiles per h2 call
CHTOK = CH * P * S                   # 4096


class CapacityError(Exception):
    pass


# ---------------------------------------------------------------- host prep

def _wrap16(lists):
    """[ncalls, n] int16 logical lists -> [128, ncalls*(n//16)]: entry j of
    call k at (partition j%16, col k*(n//16)+j//16), replicated across the 8
    groups of 16 partitions."""
    ncalls, n = lists.shape
    w = lists.reshape(ncalls, n // 16, 16).transpose(0, 2, 1)
    out = np.concatenate([w[k] for k in range(ncalls)], axis=1)
    return np.ascontiguousarray(np.tile(out, (8, 1)))


def _prep_x4(idx32):
    """idx32 [NTILES*P, S] int32 -> (h1_idx, h2_idx) wrapped int16 tensors."""
    assert idx32.shape == (NTILES * P, S)
    n = idx32.reshape(-1)
    j = np.repeat(np.arange(NTILES * P) // P, S)
    p = np.repeat(np.arange(NTILES * P) % P, S)
    s = np.tile(np.arange(S), NTILES * P)

    reg_of_tile = np.repeat(np.arange(NREG), REGION_TILES)
    tile0_of_reg = np.concatenate(([0], np.cumsum(REGION_TILES)[:-1]))
    r = reg_of_tile[j]
    plane = n >> 15
    local = (n & 32767).astype(np.int16)

    sec = plane * NREG + r
    order = np.argsort(sec, kind="stable")
    counts = np.bincount(sec, minlength=PLANES * NREG)
    if counts.max() > SEC_CAP:
        raise CapacityError(f"section overflow: {counts.max()} > {SEC_CAP}")
    starts = np.concatenate(([0], np.cumsum(counts)[:-1]))
    rank = np.empty_like(order)
    rank[order] = np.arange(order.size) - starts[sec[order]]

    h1 = np.zeros((PLANES, H1CAP), dtype=np.int16)
    h1[plane, r * SEC_CAP + rank] = local

    srow = (plane * SEC_CAP + (rank % P) * SEC_COLS + rank // P).astype(
        np.int16)
    qpos = ((j - tile0_of_reg[r]) * S + s) * P + p
    h2_list = np.zeros(NTILES * P * S, dtype=np.int16)
    reg_tok0 = np.concatenate(
        ([0], np.cumsum(np.array(REGION_TILES) * P * S)[:-1]))
    h2_list[reg_tok0[r] + qpos] = srow
    return _wrap16(h1), _wrap16(h2_list.reshape(1, -1))


def _pad_core(idx32_core):
    pad = NTILES * P - idx32_core.shape[0]
    if pad:
        idx32_core = np.concatenate([idx32_core, idx32_core[:pad]], axis=0)
    return idx32_core


def _host_prep(neigh_idx):
    idx32 = np.asarray(neigh_idx).astype(np.int32)
    return [
        _prep_x4(_pad_core(idx32[c * B_CORE : (c + 1) * B_CORE]))
        for c in range(N_CORES)
    ]


def _prep_idx(idx32):
    """per-sample fallback layout: [b_core, S] -> [P, ntiles*S] int32."""
    b_core = idx32.shape[0]
    ntiles = (b_core + P - 1) // P
    pad = ntiles * P - b_core
    if pad:
        idx32 = np.concatenate([idx32, idx32[:pad]], axis=0)
    return np.ascontiguousarray(
        idx32.reshape(ntiles, P, S).transpose(1, 0, 2).reshape(P, ntiles * S)
    )


# ------------------------------------------------------------ x4 bass kernel

def build_nc_x4(reps=1, g1bufs=2, g2bufs=3):
    nc = bacc.Bacc("TRN2", target_bir_lowering=False, num_swdge_queues=1)
    feats = nc.dram_tensor("features", [NUM_NODES, D], mybir.dt.bfloat16,
                           kind="ExternalInput")
    h1i = nc.dram_tensor("h1_idx", [P, PLANES * (H1CAP // 16)],
                         mybir.dt.int16, kind="ExternalInput")
    h2i = nc.dram_tensor("h2_idx", [P, NTILES * P * S // 16],
                         mybir.dt.int16, kind="ExternalInput")
    out = nc.dram_tensor("out", [NTILES * P, D], mybir.dt.float32,
                         kind="ExternalOutput")
    staging = nc.dram_tensor("staging", [NREG * REG_ROWS, D],
                             mybir.dt.bfloat16, kind="Internal")
    out_re = out[:].rearrange("(j p) d -> p j d", p=P)
    st_w = staging[:].rearrange(
        "(r pl p c) d -> pl p r (c d)", r=NREG, pl=PLANES, p=P, c=SEC_COLS)

    reg_tile0 = np.concatenate(([0], np.cumsum(REGION_TILES)[:-1]))

    with tile.TileContext(nc) as tc:
        with (
            tc.tile_pool(name="idxp", bufs=1) as idxp,
            tc.tile_pool(name="g1p", bufs=g1bufs) as g1p,
            tc.tile_pool(name="g2p", bufs=g2bufs) as g2p,
            tc.tile_pool(name="resp", bufs=g2bufs) as resp,
        ):
            h1_sb = idxp.tile([P, PLANES * (H1CAP // 16)], mybir.dt.int16)
            h2_sb = idxp.tile([P, NTILES * P * S // 16], mybir.dt.int16)
            nc.sync.dma_start(out=h1_sb[:], in_=h1i[:])
            nc.sync.dma_start(out=h2_sb[:], in_=h2i[:])
            probe = idxp.tile([1, 1], mybir.dt.int16)
            nc.gpsimd.load_library(mlp)
            nc.gpsimd.tensor_copy(probe[:], h1_sb[:1, :1])
            nc.gpsimd.tensor_copy(probe[:], h2_sb[:1, :1])

            def body():
                # phase 1: plane gathers -> region-major staging
                for pl in range(PLANES):
                    g1 = g1p.tile([P, (H1CAP // P) * D], mybir.dt.bfloat16)
                    nc.gpsimd.memset(g1[:1, :1], 0)
                    g1_3 = g1[:].rearrange("p (i d) -> p i d",
                                           i=H1CAP // P, d=D)
                    lo = pl * PLANE_ROWS
                    hi = min(NUM_NODES, lo + PLANE_ROWS)
                    nc.gpsimd.dma_gather(
                        g1_3, feats[lo:hi, :],
                        h1_sb[:, pl * (H1CAP // 16) : (pl + 1) * (H1CAP // 16)],
                        H1CAP, H1CAP, D, queue_num=0,
                        single_packet=False,
                    )
                    g1_w = g1[:].rearrange("p (r c d) -> p r (c d)",
                                           r=NREG, c=SEC_COLS, d=D)
                    nc.sync.dma_start(out=st_w[pl], in_=g1_w)

                # phase 2: per-chunk batch-ordered gathers + reduce
                for ci in range(NTILES // CH):
                    j0 = ci * CH
                    reg = int(np.searchsorted(reg_tile0, j0, side="right")) - 1
                    g2 = g2p.tile([P, CH * S * D], mybir.dt.bfloat16)
                    nc.gpsimd.memset(g2[:1, :1], 0)
                    g2_3 = g2[:].rearrange("p (i d) -> p i d", i=CH * S, d=D)
                    nc.gpsimd.dma_gather(
                        g2_3,
                        staging[reg * REG_ROWS : (reg + 1) * REG_ROWS, :],
                        h2_sb[:, ci * (CHTOK // 16) : (ci + 1) * (CHTOK // 16)],
                        CHTOK, CHTOK, D, queue_num=0,
                        single_packet=False,
                    )
                    g4 = g2[:].rearrange("p (c s d) -> p c s d",
                                         c=CH, s=S, d=D)
                    width = S
                    while width > 1:
                        half = width // 2
                        nc.vector.tensor_add(
                            g4[:, :, 0:half, :],
                            g4[:, :, 0:half, :],
                            g4[:, :, half : 2 * half, :],
                        )
                        width = half
                    r = resp.tile([P, CH * D], mybir.dt.float32)
                    r4 = r[:].rearrange("p (c o d) -> p c o d",
                                        c=CH, o=1, d=D)
                    nc.vector.tensor_scalar_mul(
                        r4[:, :, :, :], g4[:, :, 0:1, :], 1.0 / S)
                    nc.sync.dma_start(out=out_re[:, j0 : j0 + CH, :],
                                      in_=r[:])

            if reps == 1:
                body()
            else:
                with tc.For_i(0, reps, 1):
                    body()
    nc.compile()
    return nc


# --------------------------------------------- per-sample fallback kernel

def build_nc_persample(reps=1, n_queues=4, bufs=3):
    ntiles = NTILES
    nc = bacc.Bacc("TRN2", target_bir_lowering=False,
                   num_swdge_queues=n_queues)
    feats = nc.dram_tensor("features", [NUM_NODES, D], mybir.dt.float32,
                           kind="ExternalInput")
    idx = nc.dram_tensor("idx_t", [P, ntiles * S], mybir.dt.int32,
                         kind="ExternalInput")
    out = nc.dram_tensor("out", [ntiles * P, D], mybir.dt.float32,
                         kind="ExternalOutput")
    out_re = out[:].rearrange("(j p) d -> p j d", p=P)

    with tile.TileContext(nc) as tc:
        with (
            tc.tile_pool(name="idxp", bufs=1) as idxp,
            tc.tile_pool(name="gatp", bufs=bufs) as gatp,
            tc.tile_pool(name="resp", bufs=bufs) as resp,
        ):
            idx_sb = idxp.tile([P, ntiles * S], mybir.dt.int32)
            nc.sync.dma_start(out=idx_sb[:], in_=idx[:])
            probe = idxp.tile([1, 1], mybir.dt.int32)
            nc.gpsimd.tensor_copy(probe[:], idx_sb[:1, :1])

            def tile_body(j):
                g = gatp.tile([P, S * D], mybir.dt.float32)
                nc.gpsimd.memset(g[:1, :1], 0)
                for s in range(S):
                    inst = nc.gpsimd.indirect_dma_start(
                        out=g[:, s * D : (s + 1) * D],
                        out_offset=None,
                        in_=feats[:],
                        in_offset=bass.IndirectOffsetOnAxis(
                            ap=idx_sb[:, j * S + s : j * S + s + 1], axis=0
                        ),
                    )
                    if n_queues > 1:
                        inst.ins.queue = f"qPoolDynamic{s % n_queues or ''}"
                width = S
                while width > 1:
                    half = width // 2
                    nc.vector.tensor_add(
                        g[:, 0 : half * D],
                        g[:, 0 : half * D],
                        g[:, half * D : 2 * half * D],
                    )
                    width = half
                r = resp.tile([P, D], mybir.dt.float32)
                nc.vector.tensor_scalar_mul(r[:], g[:, 0:D], 1.0 / S)
                nc.sync.dma_start(out=out_re[:, j, :], in_=r[:])

            if reps == 1:
                for j in range(ntiles):
                    tile_body(j)
            else:
                with tc.For_i(0, reps, 1):
                    for j in range(ntiles):
                        tile_body(j)
    nc.compile()
    return nc


_nc_cache = {}


def _get(builder, **kw):
    key = (builder.__name__, tuple(sorted(kw.items())))
    if key not in _nc_cache:
        _nc_cache[key] = builder(**kw)
    return _nc_cache[key]


def _run_x4(features, neigh_idx):
    prep = _host_prep(neigh_idx)          # raises CapacityError on overflow
    feats_bf = np.ascontiguousarray(
        np.asarray(features, dtype=np.float32).astype(ml_dtypes.bfloat16))
    nc = _get(build_nc_x4)
    in_maps = [
        {"features": feats_bf, "h1_idx": prep[c][0], "h2_idx": prep[c][1]}
        for c in range(N_CORES)
    ]
    res = run_bass_kernel_spmd(nc, in_maps, core_ids=list(range(N_CORES)))
    return np.concatenate([r["out"][:B_CORE] for r in res.results], axis=0)


def _run_persample(features, neigh_idx):
    features = np.ascontiguousarray(features, dtype=np.float32)
    idx32 = np.asarray(neigh_idx).astype(np.int32)
    nc = _get(build_nc_persample)
    in_maps = [
        {
            "features": features,
            "idx_t": _prep_idx(idx32[c * B_CORE : (c + 1) * B_CORE]),
        }
        for c in range(N_CORES)
    ]
    res = run_bass_kernel_spmd(nc, in_maps, core_ids=list(range(N_CORES)))
    return np.concatenate([r["out"][:B_CORE] for r in res.results], axis=0)


def kernel(features, neigh_idx, num_sample):
    assert np.asarray(features).shape == (NUM_NODES, D)
    assert np.asarray(neigh_idx).shape == (BATCH, S)
    return _run_persample(features, neigh_idx)


# bench alias
def build_nc(reps=1, n_queues=4, bufs=3, **_ignored):
    return build_nc_persample(reps=reps, n_queues=n_queues, bufs=bufs)


# revision 9
# speedup vs baseline: 2.2616x; 1.0355x over previous
"""MeanAggregator (GNN mean message passing) on 8 Trainium2 NeuronCores.

reference:
    neigh_feats = features[neigh_idx]          # [batch, num_sample, d_feat]
    out = mean(neigh_feats, axis=1)            # [batch, d_feat]

Shapes (hardcoded): features [1_000_000, 128] f32, neigh_idx [100_000, 16] i64.

Data-parallel over the batch across 8 cores (12_500 rows each), features
replicated (host-cast to bf16: rel tolerance 2e-2 >> bf16 error).

Default path = radix-staged bulk gather ("x4"): the only bulk-descriptor
gather primitive (InstDMAGatherAnt / gpsimd.dma_gather) takes int16 indices
(reach 32767 rows x 256 B), so the 256 MB bf16 table is covered by 31
"plane" calls -- which scrambles output order (each call writes consecutive
SBUF slots). A 2-pass radix fixes the order with all-static capacities:

  h1 (per plane): dma_gather the plane's sampled rows in REGION-sorted order
      (region = fixed run of batch tiles; fixed-capacity per-(plane, region)
      sections, dummy-padded), then one static HWDGE DMA into a DRAM staging
      area laid out region-major.
  h2 (per 2-tile chunk): dma_gather from that region's staging block
      (<= 32767 rows, int16-addressable) in exact batch-interleaved order:
      token q' = (tile_local*16 + s)*128 + p lands at partition p, col
      tile_local*16 + s, so one batch row's 16 samples are 16 consecutive
      256 B slots on its own partition. DVE tree-reduce, scale 1/16 with f32
      output, DMA out.

~1 us SWDGE fixed cost is paid ~80x per rep instead of 1568x (the per-sample
indirect-DMA fallback below, used only on section-capacity overflow --
P < 1e-6 for uniform indices -- or if the x4 build fails).
"""

import numpy as np
import ml_dtypes

import concourse.bacc as bacc
import concourse.bass as bass
import concourse.mybir as mybir
import concourse.tile as tile
from concourse.library_config import mlp
from concourse.bass_utils import run_bass_kernel_spmd

N_CORES = 8
P = 128
D = 128
S = 16
NUM_NODES = 1_000_000
BATCH = 100_000
B_CORE = BATCH // N_CORES            # 12500
NTILES = (B_CORE + P - 1) // P       # 98

PLANE_ROWS = 32768                   # int16 reach per dma_gather call
PLANES = (NUM_NODES + PLANE_ROWS - 1) // PLANE_ROWS   # 31
REGION_TILES = [12] * 8 + [2]        # sum = 98, all even (CH divides each)
NREG = len(REGION_TILES)
SEC_CAP = 1024                       # tokens per (plane, region) section
SEC_COLS = SEC_CAP // P              # 8
H1CAP = NREG * SEC_CAP               # 9216 tokens per plane call
REG_ROWS = PLANES * SEC_CAP          # 31744 staging rows/region (<= 32767)
CH = 2                               # tiles per h2 call
CHTOK = CH * P * S                   # 4096


class CapacityError(Exception):
    pass


# ---------------------------------------------------------------- host prep

def _wrap16(lists):
    """[ncalls, n] int16 logical lists -> [128, ncalls*(n//16)]: entry j of
    call k at (partition j%16, col k*(n//16)+j//16), replicated across the 8
    groups of 16 partitions."""
    ncalls, n = lists.shape
    w = lists.reshape(ncalls, n // 16, 16).transpose(0, 2, 1)
    out = np.concatenate([w[k] for k in range(ncalls)], axis=1)
    return np.ascontiguousarray(np.tile(out, (8, 1)))


def _prep_x4(idx32):
    """idx32 [NTILES*P, S] int32 -> (h1_idx, h2_idx) wrapped int16 tensors."""
    assert idx32.shape == (NTILES * P, S)
    n = idx32.reshape(-1)
    j = np.repeat(np.arange(NTILES * P) // P, S)
    p = np.repeat(np.arange(NTILES * P) % P, S)
    s = np.tile(np.arange(S), NTILES * P)

    reg_of_tile = np.repeat(np.arange(NREG), REGION_TILES)
    tile0_of_reg = np.concatenate(([0], np.cumsum(REGION_TILES)[:-1]))
    r = reg_of_tile[j]
    plane = n >> 15
    local = (n & 32767).astype(np.int16)

    sec = plane * NREG + r
    order = np.argsort(sec, kind="stable")
    counts = np.bincount(sec, minlength=PLANES * NREG)
    if counts.max() > SEC_CAP:
        raise CapacityError(f"section overflow: {counts.max()} > {SEC_CAP}")
    starts = np.concatenate(([0], np.cumsum(counts)[:-1]))
    rank = np.empty_like(order)
    rank[order] = np.arange(order.size) - starts[sec[order]]

    h1 = np.zeros((PLANES, H1CAP), dtype=np.int16)
    h1[plane, r * SEC_CAP + rank] = local

    srow = (plane * SEC_CAP + (rank % P) * SEC_COLS + rank // P).astype(
        np.int16)
    qpos = ((j - tile0_of_reg[r]) * S + s) * P + p
    h2_list = np.zeros(NTILES * P * S, dtype=np.int16)
    reg_tok0 = np.concatenate(
        ([0], np.cumsum(np.array(REGION_TILES) * P * S)[:-1]))
    h2_list[reg_tok0[r] + qpos] = srow
    return _wrap16(h1), _wrap16(h2_list.reshape(1, -1))


def _pad_core(idx32_core):
    pad = NTILES * P - idx32_core.shape[0]
    if pad:
        idx32_core = np.concatenate([idx32_core, idx32_core[:pad]], axis=0)
    return idx32_core


def _host_prep(neigh_idx):
    idx32 = np.asarray(neigh_idx).astype(np.int32)
    return [
        _prep_x4(_pad_core(idx32[c * B_CORE : (c + 1) * B_CORE]))
        for c in range(N_CORES)
    ]


def _prep_idx(idx32):
    """per-sample fallback layout: [b_core, S] -> [P, ntiles*S] int32."""
    b_core = idx32.shape[0]
    ntiles = (b_core + P - 1) // P
    pad = ntiles * P - b_core
    if pad:
        idx32 = np.concatenate([idx32, idx32[:pad]], axis=0)
    return np.ascontiguousarray(
        idx32.reshape(ntiles, P, S).transpose(1, 0, 2).reshape(P, ntiles * S)
    )


# ------------------------------------------------------------ x4 bass kernel

def build_nc_x4(reps=1, g1bufs=2, g2bufs=3):
    nc = bacc.Bacc("TRN2", target_bir_lowering=False, num_swdge_queues=1)
    feats = nc.dram_tensor("features", [NUM_NODES, D], mybir.dt.bfloat16,
                           kind="ExternalInput")
    h1i = nc.dram_tensor("h1_idx", [P, PLANES * (H1CAP // 16)],
                         mybir.dt.int16, kind="ExternalInput")
    h2i = nc.dram_tensor("h2_idx", [P, NTILES * P * S // 16],
                         mybir.dt.int16, kind="ExternalInput")
    out = nc.dram_tensor("out", [NTILES * P, D], mybir.dt.float32,
                         kind="ExternalOutput")
    staging = nc.dram_tensor("staging", [NREG * REG_ROWS, D],
                             mybir.dt.bfloat16, kind="Internal")
    out_re = out[:].rearrange("(j p) d -> p j d", p=P)
    st_w = staging[:].rearrange(
        "(r pl p c) d -> pl p r (c d)", r=NREG, pl=PLANES, p=P, c=SEC_COLS)

    reg_tile0 = np.concatenate(([0], np.cumsum(REGION_TILES)[:-1]))

    with tile.TileContext(nc) as tc:
        with (
            tc.tile_pool(name="idxp", bufs=1) as idxp,
            tc.tile_pool(name="g1p", bufs=g1bufs) as g1p,
            tc.tile_pool(name="g2p", bufs=g2bufs) as g2p,
            tc.tile_pool(name="resp", bufs=g2bufs) as resp,
        ):
            h1_sb = idxp.tile([P, PLANES * (H1CAP // 16)], mybir.dt.int16)
            h2_sb = idxp.tile([P, NTILES * P * S // 16], mybir.dt.int16)
            nc.sync.dma_start(out=h1_sb[:], in_=h1i[:])
            nc.sync.dma_start(out=h2_sb[:], in_=h2i[:])
            probe = idxp.tile([1, 1], mybir.dt.int16)
            nc.gpsimd.load_library(mlp)
            nc.gpsimd.tensor_copy(probe[:], h1_sb[:1, :1])
            nc.gpsimd.tensor_copy(probe[:], h2_sb[:1, :1])

            def body():
                # phase 1: plane gathers -> region-major staging
                for pl in range(PLANES):
                    g1 = g1p.tile([P, (H1CAP // P) * D], mybir.dt.bfloat16)
                    nc.gpsimd.memset(g1[:1, :1], 0)
                    g1_3 = g1[:].rearrange("p (i d) -> p i d",
                                           i=H1CAP // P, d=D)
                    lo = pl * PLANE_ROWS
                    hi = min(NUM_NODES, lo + PLANE_ROWS)
                    nc.gpsimd.dma_gather(
                        g1_3, feats[lo:hi, :],
                        h1_sb[:, pl * (H1CAP // 16) : (pl + 1) * (H1CAP // 16)],
                        H1CAP, H1CAP, D, queue_num=0,
                        single_packet=False,
                    )
                    g1_w = g1[:].rearrange("p (r c d) -> p r (c d)",
                                           r=NREG, c=SEC_COLS, d=D)
                    nc.sync.dma_start(out=st_w[pl], in_=g1_w)

                # phase 2: per-chunk batch-ordered gathers + reduce
                for ci in range(NTILES // CH):
                    j0 = ci * CH
                    reg = int(np.searchsorted(reg_tile0, j0, side="right")) - 1
                    g2 = g2p.tile([P, CH * S * D], mybir.dt.bfloat16)
                    nc.gpsimd.memset(g2[:1, :1], 0)
                    g2_3 = g2[:].rearrange("p (i d) -> p i d", i=CH * S, d=D)
                    nc.gpsimd.dma_gather(
                        g2_3,
                        staging[reg * REG_ROWS : (reg + 1) * REG_ROWS, :],
                        h2_sb[:, ci * (CHTOK // 16) : (ci + 1) * (CHTOK // 16)],
                        CHTOK, CHTOK, D, queue_num=0,
                        single_packet=False,
                    )
                    g4 = g2[:].rearrange("p (c s d) -> p c s d",
                                         c=CH, s=S, d=D)
                    width = S
                    while width > 1:
                        half = width // 2
                        nc.vector.tensor_add(
                            g4[:, :, 0:half, :],
                            g4[:, :, 0:half, :],
                            g4[:, :, half : 2 * half, :],
                        )
                        width = half
                    r = resp.tile([P, CH * D], mybir.dt.float32)
                    r4 = r[:].rearrange("p (c o d) -> p c o d",
                                        c=CH, o=1, d=D)
                    nc.vector.tensor_scalar_mul(
                        r4[:, :, :, :], g4[:, :, 0:1, :], 1.0 / S)
                    nc.sync.dma_start(out=out_re[:, j0 : j0 + CH, :],
                                      in_=r[:])

            if reps == 1:
                body()
            else:
                with tc.For_i(0, reps, 1):
                    body()
    nc.compile()
    return nc


# --------------------------------------------- per-sample fallback kernel

def build_nc_persample(reps=1, n_queues=4, bufs=6):
    ntiles = NTILES
    nc = bacc.Bacc("TRN2", target_bir_lowering=False,
                   num_swdge_queues=n_queues)
    feats = nc.dram_tensor("features", [NUM_NODES, D], mybir.dt.float32,
                           kind="ExternalInput")
    idx = nc.dram_tensor("idx_t", [P, ntiles * S], mybir.dt.int32,
                         kind="ExternalInput")
    out = nc.dram_tensor("out", [ntiles * P, D], mybir.dt.float32,
                         kind="ExternalOutput")
    out_re = out[:].rearrange("(j p) d -> p j d", p=P)

    with tile.TileContext(nc) as tc:
        with (
            tc.tile_pool(name="idxp", bufs=1) as idxp,
            tc.tile_pool(name="gatp", bufs=bufs) as gatp,
            tc.tile_pool(name="resp", bufs=bufs) as resp,
        ):
            idx_sb = idxp.tile([P, ntiles * S], mybir.dt.int32)
            nc.sync.dma_start(out=idx_sb[:], in_=idx[:])
            probe = idxp.tile([1, 1], mybir.dt.int32)
            nc.gpsimd.tensor_copy(probe[:], idx_sb[:1, :1])

            def tile_body(j):
                g = gatp.tile([P, S * D], mybir.dt.float32)
                nc.gpsimd.memset(g[:1, :1], 0)
                for s in range(S):
                    inst = nc.gpsimd.indirect_dma_start(
                        out=g[:, s * D : (s + 1) * D],
                        out_offset=None,
                        in_=feats[:],
                        in_offset=bass.IndirectOffsetOnAxis(
                            ap=idx_sb[:, j * S + s : j * S + s + 1], axis=0
                        ),
                    )
                    if n_queues > 1:
                        inst.ins.queue = f"qPoolDynamic{s % n_queues or ''}"
                width = S
                while width > 1:
                    half = width // 2
                    nc.vector.tensor_add(
                        g[:, 0 : half * D],
                        g[:, 0 : half * D],
                        g[:, half * D : 2 * half * D],
                    )
                    width = half
                r = resp.tile([P, D], mybir.dt.float32)
                nc.vector.tensor_scalar_mul(r[:], g[:, 0:D], 1.0 / S)
                nc.sync.dma_start(out=out_re[:, j, :], in_=r[:])

            if reps == 1:
                for j in range(ntiles):
                    tile_body(j)
            else:
                with tc.For_i(0, reps, 1):
                    for j in range(ntiles):
                        tile_body(j)
    nc.compile()
    return nc


_nc_cache = {}


def _get(builder, **kw):
    key = (builder.__name__, tuple(sorted(kw.items())))
    if key not in _nc_cache:
        _nc_cache[key] = builder(**kw)
    return _nc_cache[key]


def _run_x4(features, neigh_idx):
    prep = _host_prep(neigh_idx)          # raises CapacityError on overflow
    feats_bf = np.ascontiguousarray(
        np.asarray(features, dtype=np.float32).astype(ml_dtypes.bfloat16))
    nc = _get(build_nc_x4)
    in_maps = [
        {"features": feats_bf, "h1_idx": prep[c][0], "h2_idx": prep[c][1]}
        for c in range(N_CORES)
    ]
    res = run_bass_kernel_spmd(nc, in_maps, core_ids=list(range(N_CORES)))
    return np.concatenate([r["out"][:B_CORE] for r in res.results], axis=0)


def _run_persample(features, neigh_idx):
    features = np.ascontiguousarray(features, dtype=np.float32)
    idx32 = np.asarray(neigh_idx).astype(np.int32)
    nc = _get(build_nc_persample)
    in_maps = [
        {
            "features": features,
            "idx_t": _prep_idx(idx32[c * B_CORE : (c + 1) * B_CORE]),
        }
        for c in range(N_CORES)
    ]
    res = run_bass_kernel_spmd(nc, in_maps, core_ids=list(range(N_CORES)))
    return np.concatenate([r["out"][:B_CORE] for r in res.results], axis=0)


def kernel(features, neigh_idx, num_sample):
    assert np.asarray(features).shape == (NUM_NODES, D)
    assert np.asarray(neigh_idx).shape == (BATCH, S)
    return _run_persample(features, neigh_idx)


# bench alias
def build_nc(reps=1, n_queues=4, bufs=6, **_ignored):
    return build_nc_persample(reps=reps, n_queues=n_queues, bufs=bufs)
